# revision 10
# baseline (speedup 1.0000x reference)
"""Trainium2 Bass kernel for CSAttention.

Reference computation (per batch b of 32, N=1024 tokens, C=512 channels,
L=512 latent):
    qk  = x @ W_qk.T + b_qk            # [N, 2L]
    q   = qk[:, :L] * L**-0.5
    k   = qk[:, L:]
    out = softmax(q @ k.T, -1) @ y     # [N, C]

Sharding: data-parallel over the batch axis across 8 NeuronCores
(4 batches per core); W_qk / b_qk replicated.

Per-core kernel structure (bf16 matmul operands, fp32 PSUM accumulate):
  load:    x/y/W stream in via gpsimd casting DMA (f32 dram -> bf16 sbuf);
           x and W are transposed on the PE in groups of 4 tiles sharing
           one PSUM tile, drained by a single wide copy (the copies are
           overhead-dominated, so 1x512 beats 4x128 by ~2x). Steady-state
           transpose groups ride between stage C chains so the PE never
           has a sparse window (the HAM clock gate halves the PE clock
           whenever a ~3.4us activity window looks idle).
  warmup:  a short burst of dummy matmuls runs during the initial DMA
           wait so the PE HAM clock gate reaches 2.4 GHz before real work
  stage B: QKT[l, n] = (WT col-slice).T @ XT          (+bias)   [2L, N]
  stage C: ST[m, n]  = (KT col-slice).T @ QT   -> exp(scale*.)  [N, N]
  stage D: out[n, :] = (ET col-slice).T @ [Y | 1]; the appended ones
           column produces the softmax denominator inside the same PSUM
           accumulation; normalize with DVE reciprocal + per-partition mul.
"""

import numpy as np

import concourse.bass as bass
import concourse.mybir as mybir
import concourse.tile as tile
from concourse import bacc
from concourse.bass_utils import run_bass_kernel_spmd
from concourse.masks import make_identity

P = 128
N_CORES = 8
B_FULL = 32
B_PER_CORE = B_FULL // N_CORES  # 4
N = 1024            # tokens
C = 512             # channels
L = 512             # latent
TWO_L = 2 * L
NT = N // P         # 8 token tiles
CT = C // P         # 4 channel tiles
LT = TWO_L // P     # 8 latent tiles (0..3 = q, 4..7 = k)
SCALE = float(L) ** -0.5
YA = C + 1          # augmented Y width: [Y | ones]
NA = 257            # first stage-D matmul free dim
NB = YA - NA        # 256
SCOL = C - NA       # ones column's index within psum_B (= 255)
N_WARM = 9          # dummy matmuls to release the HAM clock gate

F32 = mybir.dt.float32
BF16 = mybir.dt.bfloat16
IDENT = mybir.ActivationFunctionType.Identity
EXP = mybir.ActivationFunctionType.Exp


def _emit(tc, x, y, w, bvec, out):
    """Emit the per-core kernel. x/y: [B_PER_CORE, N, C] dram APs,
    w: [2L, C], bvec: [2L], out: [B_PER_CORE, N, C]."""
    from contextlib import ExitStack

    nc = tc.nc
    with ExitStack() as ctx:
        const = ctx.enter_context(tc.tile_pool(name="const", bufs=1))
        big = ctx.enter_context(tc.tile_pool(name="big", bufs=1))
        outp = ctx.enter_context(tc.tile_pool(name="outp", bufs=4))
        rsp = ctx.enter_context(tc.tile_pool(name="rsp", bufs=4))
        ps_mm = ctx.enter_context(tc.tile_pool(name="ps_mm", bufs=3, space="PSUM"))
        ps_d = ctx.enter_context(tc.tile_pool(name="ps_d", bufs=3, space="PSUM"))
        ps_tr = ctx.enter_context(tc.tile_pool(name="ps_tr", bufs=2, space="PSUM"))

        # b_qk striped so partition p, col t  <-  b_qk[t*128 + p]
        bias_sb = const.tile([P, LT], F32, tag="bias")
        nc.sync.dma_start(bias_sb, bvec.rearrange("(o p) -> p o", p=P))

        # PE warm-up: the HAM clock gate defaults to 1.2 GHz and releases
        # to 2.4 GHz only after ~3.4us of sustained activity. Burn that
        # window on dummy matmuls while the first loads are in flight.
        warm_sb = const.tile([P, 512], BF16, tag="warm")
        nc.vector.memset(warm_sb, 0.0)
        ps_warm = ps_mm.tile([P, 512], F32, tag="mm")
        for k in range(N_WARM):
            nc.tensor.matmul(
                ps_warm, warm_sb[:, 0:P], warm_sb,
                start=(k == 0), stop=(k == N_WARM - 1),
            )

        identity = const.tile([P, P], BF16, tag="ident")
        make_identity(nc, identity)

        # ---- persistent workspaces ----
        wbf = big.tile([P, LT, C], BF16, tag="wbf")
        wt = big.tile([P, CT, TWO_L], BF16, tag="wt")
        w_tiled = w.rearrange("(t p) c -> p t c", p=P)
        xbf2 = [
            big.tile([P, NT, C], BF16, tag=f"xbf{j}", name=f"xbf{j}")
            for j in range(2)
        ]
        xt2 = [
            big.tile([P, CT, N], BF16, tag=f"xt{j}", name=f"xt{j}")
            for j in range(2)
        ]
        qkt = big.tile([P, LT, N], BF16, tag="qkt")     # rows l, cols n
        et = big.tile([P, NT, N], BF16, tag="et")       # rows m, cols n
        yaug = big.tile([P, NT, YA], BF16, tag="yaug")  # [Y | ones | pad]
        nc.vector.memset(yaug[:, :, C:YA], 1.0)

        def tr_x_group(i, nt_, copy_eng=None):
            """Transpose all 4 (nt_, ct) x tiles via one PSUM tile + one
            wide strided copy-back."""
            src, dst = xbf2[i % 2], xt2[i % 2]
            # padded to a full 2 KiB PSUM bank so the pool's two bufs never
            # share a bank (PE-write + engine-read of one bank is a race)
            ps = ps_tr.tile([P, CT, 2 * P], BF16, tag="tr")
            for ct in range(CT):
                nc.tensor.transpose(
                    ps[:, ct, 0:P], src[:, nt_, ct * P:(ct + 1) * P], identity
                )
            eng = copy_eng or nc.vector
            if eng is nc.vector:
                nc.vector.tensor_copy(
                    dst[:, :, nt_ * P:(nt_ + 1) * P], ps[:, :, 0:P]
                )
            else:
                eng.activation(
                    dst[:, :, nt_ * P:(nt_ + 1) * P], ps[:, :, 0:P], IDENT
                )

        def tr_w_group(lt, copy_eng=None):
            ps = ps_tr.tile([P, CT, 2 * P], BF16, tag="tr")
            for ct in range(CT):
                nc.tensor.transpose(
                    ps[:, ct, 0:P], wbf[:, lt, ct * P:(ct + 1) * P], identity
                )
            eng = copy_eng or nc.vector
            if eng is nc.vector:
                nc.vector.tensor_copy(
                    wt[:, :, lt * P:(lt + 1) * P], ps[:, :, 0:P]
                )
            else:
                eng.activation(
                    wt[:, :, lt * P:(lt + 1) * P], ps[:, :, 0:P], IDENT
                )

        def load_x(i, chunks):
            x_tiled = x[i].rearrange("(t p) c -> p t c", p=P)
            for a, b in chunks:
                nc.gpsimd.dma_start(xbf2[i % 2][:, a:b], x_tiled[:, a:b])

        def load_y(i):
            nc.gpsimd.dma_start(
                yaug[:, :, 0:C], y[i].rearrange("(t p) c -> p t c", p=P)
            )

        def stage_b_chain(i, nh, lt):
            xt = xt2[i % 2]
            ps = ps_mm.tile([P, 512], F32, tag="mm")
            for ct in range(CT):
                nc.tensor.matmul(
                    ps,
                    wt[:, ct, lt * P:(lt + 1) * P],
                    xt[:, ct, nh * 512:(nh + 1) * 512],
                    start=(ct == 0),
                    stop=(ct == CT - 1),
                )
            nc.scalar.activation(
                qkt[:, lt, nh * 512:(nh + 1) * 512],
                ps,
                IDENT,
                bias=bias_sb[:, lt:lt + 1],
            )

        # initial loads: x0 and W interleaved in 0.5 MB chunks, ordered so
        # each lands just before its transpose group needs it.
        load_x(0, [(0, 2)])
        nc.gpsimd.dma_start(wbf[:, 0:2], w_tiled[:, 0:2])
        load_x(0, [(2, 4)])
        nc.gpsimd.dma_start(wbf[:, 2:4], w_tiled[:, 2:4])
        load_x(0, [(4, 6)])
        nc.gpsimd.dma_start(wbf[:, 4:6], w_tiled[:, 4:6])
        load_x(0, [(6, 8)])
        nc.gpsimd.dma_start(wbf[:, 6:8], w_tiled[:, 6:8])

        # prologue: transpose x0 first half (copy engines alternate), then
        # interleave W / x0-second-half transpose groups into stage B's
        # chains so the PE stream stays dense for the HAM gate.
        for nt_ in range(4):
            tr_x_group(0, nt_, copy_eng=nc.scalar if nt_ % 2 else nc.vector)
        for lt in range(LT):
            tr_w_group(lt, copy_eng=nc.scalar if lt % 2 else nc.vector)
            stage_b_chain(0, 0, lt)
            if lt < 4:
                tr_x_group(0, 4 + lt,
                           copy_eng=nc.vector if lt % 2 else nc.scalar)
        for lt in range(LT):
            stage_b_chain(0, 1, lt)

        for i in range(B_PER_CORE):
            first = i == 0
            if i + 1 < B_PER_CORE:
                load_x(i + 1, [(0, 4), (4, 8)])
            load_y(i)

            if not first:
                for nh in range(2):
                    for lt in range(LT):
                        stage_b_chain(i, nh, lt)

            # ---- stage C: ST[m, n] = K[m] . Q[n] ; ET = exp(scale * ST) ----
            # next batch's x transpose groups ride between the chains.
            ti = 0
            for nh in range(2):
                for mt in range(NT):
                    ps = ps_mm.tile([P, 512], F32, tag="mm")
                    for lq in range(4):
                        nc.tensor.matmul(
                            ps,
                            qkt[:, 4 + lq, mt * P:(mt + 1) * P],
                            qkt[:, lq, nh * 512:(nh + 1) * 512],
                            start=(lq == 0),
                            stop=(lq == 3),
                        )
                    nc.scalar.activation(
                        et[:, mt, nh * 512:(nh + 1) * 512], ps, EXP, scale=SCALE
                    )
                    if i + 1 < B_PER_CORE and mt % 2 == 1:
                        tr_x_group(i + 1, ti)
                        ti += 1

            # ---- stage D: out = ET.T @ [Y | 1], then normalize ----
            for nt_ in range(NT):
                psA = ps_d.tile([P, NA], F32, tag="d")
                psB = ps_d.tile([P, NA], F32, tag="d")
                for mt in range(NT):
                    lw = et[:, mt, nt_ * P:(nt_ + 1) * P]
                    nc.tensor.matmul(
                        psA, lw, yaug[:, mt, 0:NA],
                        start=(mt == 0), stop=(mt == NT - 1),
                    )
                    nc.tensor.matmul(
                        psB[:, 0:NB], lw, yaug[:, mt, NA:YA],
                        start=(mt == 0), stop=(mt == NT - 1),
                    )
                rs = rsp.tile([P, 1], F32, tag="rs")
                nc.vector.reciprocal(rs, psB[:, SCOL:SCOL + 1])
                ob = outp.tile([P, C], F32, tag="ob")
                nc.scalar.mul(ob[:, 0:NA], psA[:, 0:NA], rs)
                nc.vector.tensor_scalar_mul(ob[:, NA:C], psB[:, 0:SCOL], rs)
                if i == B_PER_CORE - 1 and nt_ == NT - 1:
                    # final tile: store each half as soon as its engine
                    # finishes, shortening the post-compute tail
                    nc.sync.dma_start(
                        out[i, nt_ * P:(nt_ + 1) * P, NA:C], ob[:, NA:C]
                    )
                    nc.sync.dma_start(
                        out[i, nt_ * P:(nt_ + 1) * P, 0:NA], ob[:, 0:NA]
                    )
                else:
                    nc.sync.dma_start(out[i, nt_ * P:(nt_ + 1) * P, :], ob)


_NC_CACHE = {}


def _build():
    if "nc" in _NC_CACHE:
        return _NC_CACHE["nc"]
    nc = bacc.Bacc(
        "TRN2",
        target_bir_lowering=False,
        debug=False,
        enable_asserts=False,
        num_devices=N_CORES,
    )
    x = nc.dram_tensor("x", [B_PER_CORE, N, C], F32, kind="ExternalInput").ap()
    y = nc.dram_tensor("y", [B_PER_CORE, N, C], F32, kind="ExternalInput").ap()
    w = nc.dram_tensor("W_qk", [TWO_L, C], F32, kind="ExternalInput").ap()
    bvec = nc.dram_tensor("b_qk", [TWO_L], F32, kind="ExternalInput").ap()
    out = nc.dram_tensor("out", [B_PER_CORE, N, C], F32, kind="ExternalOutput").ap()
    with tile.TileContext(nc) as tc:
        _emit(tc, x, y, w, bvec, out)
    nc.compile()
    _NC_CACHE["nc"] = nc
    return nc


def run(x, y, W_qk, b_qk, trace=False):
    """Run the SPMD kernel on 8 cores; returns (out, BassKernelResults)."""
    nc = _build()
    x = np.ascontiguousarray(x, dtype=np.float32)
    y = np.ascontiguousarray(y, dtype=np.float32)
    W_qk = np.ascontiguousarray(W_qk, dtype=np.float32)
    b_qk = np.ascontiguousarray(b_qk, dtype=np.float32)
    in_maps = [
        {
            "x": x[k * B_PER_CORE:(k + 1) * B_PER_CORE],
            "y": y[k * B_PER_CORE:(k + 1) * B_PER_CORE],
            "W_qk": W_qk,
            "b_qk": b_qk,
        }
        for k in range(N_CORES)
    ]
    res = run_bass_kernel_spmd(
        nc, in_maps, core_ids=list(range(N_CORES)), trace=trace
    )
    outs = [r["out"] for r in res.results]
    return np.concatenate(outs, axis=0), res


def kernel(x, y, W_qk, b_qk):
    out, _ = run(x, y, W_qk, b_qk)
    return out
